# revision 8
# baseline (speedup 1.0000x reference)
"""Trainium2 Bass kernel for nn_MixtureAttention.

Math: the reference builds a (c,c) pairwise Cauchy-product matrix per batch,
row-normalizes it, and keeps only the diagonal.  `pi` cancels; with
    D[i,j] = prod_d (sig_id^2 + (mu_id - mu_jd)^2)
the kept diagonal is coef[i] = invps2[i] / (invps2[i] + sum_{j!=i} 1/D[i,j])
where invps2 = 1/prod_d sig^2, and y[b,ch,c] = x[b,ch] * coef[b,c].

Kernel strategy (8 cores; core k: batch k//2, c-rows [(k%2)*2048, +2048)):
  - PE: D factorizes per dim-pair into rank-9 bilinear forms in host-computed
    factor matrices (centered coords keep all terms <= 1).  Each pair product
    q01/q23 is ONE bf16 matmul with an exact 3-way hi/mid/lo mantissa split
    folded into the contraction dim (K=9x6=54) -- fp32-grade precision at
    bf16 speed (1 cycle/row).  A bf16 identity "spike" matmul adds
    1e15 to diagonal entries so j=i drops out of the device sum; the exact
    diagonal is re-added in the epilogue from host-computed invps2.
  - ACT: R01 = 1/q01 via Reciprocal (PSUM->SBUF), doubling as PSUM evacuation.
  - DVE: one fused custom op per element: accum += R01 * recip_1NR(q23_psum)
    (bitflip seed + 1 Newton step, ~0.36% max; errors average out in the sum).
  - Epilogue per 512-column group: coef = invps2/(Racc+invps2), transposed via
    a DRAM bounce, then y = x (x) coef as K=1 fp32 outer-product matmuls.
"""

import numpy as np
import re

B, C, D, CH = 4, 4096, 4, 256
NCORES = 8
CW = C // 2            # 2048 c-rows per core (2 cores per batch)
NBLK = CW // 128       # 16 row blocks
KSP = 54               # split contraction: 9 rank-terms x 6 hi/mid/lo pairs
NTD = C // 1024        # 4 j double-tiles per row block
NG = 4                 # epilogue groups (4 row blocks each)

_cache = {}

RECIP_C0 = -0.23549792
RECIP_C1 = 2.0017324


def _split_concat(U, V):
    """U [9,M], V [9,N] fp32 -> K=54 bf16 concatenation.  u = h+m+l is an
    EXACT 3-way bf16 mantissa split (8+8+8 bits); keeping the 6 cross-term
    blocks up to O(2^-24) gives fp32-grade products at bf16 matmul speed."""
    import ml_dtypes

    def split3(x):
        x = np.asarray(x, np.float32)
        h = x.astype(ml_dtypes.bfloat16).astype(np.float32)
        r = (x - h).astype(np.float32)
        m = r.astype(ml_dtypes.bfloat16).astype(np.float32)
        l = (r - m).astype(np.float32).astype(ml_dtypes.bfloat16).astype(np.float32)
        return h, m, l

    Uh, Um, Ul = split3(U)
    Vh, Vm, Vl = split3(V)
    Uc = np.concatenate([Uh, Uh, Um, Uh, Ul, Um], axis=0)
    Vc = np.concatenate([Vh, Vm, Vh, Vl, Vh, Vm], axis=0)
    return (np.ascontiguousarray(Uc.astype(ml_dtypes.bfloat16)),
            np.ascontiguousarray(Vc.astype(ml_dtypes.bfloat16)))


def _get_mulrecip_op():
    """Custom DVE op: out = in0 * recip_1NR(in1); accum_out = row-sum(out)."""
    if "mro" in _cache:
        return _cache["mro"]
    from operator import add
    from concourse import dve_ops as DO
    from concourse.dve_spec import AluOp, Bin, Spec, Src0, Src1, Zero, C0, C1

    name = "MUL_RECIP1NR_ACC_ANT"
    _not = Bin(AluOp.BITWISE_NOT, Src1, Src1)
    _y0 = _not * C0
    _y1 = _y0 * (C1 - Src1 * _y0)

    def _ref(in0, in1, c0, c1, c2):
        nx = (~np.asarray(in1, np.float32).view(np.int32)).view(np.float32)
        y0 = (nx * np.float32(c0)).astype(np.float32)
        y1 = (y0 * (np.float32(c1) - in1 * y0).astype(np.float32)).astype(np.float32)
        b = (in0 * y1).astype(np.float32)
        return b, b.reshape(b.shape[0], -1).sum(axis=-1, keepdims=True)

    spec = Spec(body=Src0 * _y1, accum=add, accum_init=Zero, reference=_ref)
    shas = {}
    for ver in ("v3", "v4"):
        probe = DO.DveOp(name + "_PROBE", spec, subdim=False, uops_sha={})
        if name + "_PROBE" not in DO._SUB_OPCODE_FOR_NAME:
            DO._SUB_OPCODE_FOR_NAME[name + "_PROBE"] = 0x1F
        try:
            probe.compile(ver)
        except ValueError as e:
            m = re.search(r'"(?:v3|v4)"\]="([0-9a-f]+)"', str(e))
            if not m:
                raise
            shas[ver] = m.group(1)
    op = DO.DveOp(name, spec, subdim=False, uops_sha=shas)
    if name not in DO._SUB_OPCODE_FOR_NAME:
        DO.OPS.append(op)
        DO._SUB_OPCODE_FOR_NAME[name] = DO._CUSTOM_DVE_ROW_BASE + len(DO.OPS) - 1
        assert DO._SUB_OPCODE_FOR_NAME[name] < 0x20
    DO.CUSTOM_DVE_SPECS[name] = spec
    _cache["mro"] = op
    return op


def _build(bench_nrep=None, bench_span="full"):
    import concourse.bacc as bacc
    import concourse.mybir as mybir
    from concourse.tile import TileContext

    f32 = mybir.dt.float32
    bf16 = mybir.dt.bfloat16
    Alu = mybir.AluOpType
    mro = _get_mulrecip_op()

    nc = bacc.Bacc(None, target_bir_lowering=False)
    u01T = nc.declare_dram_parameter("u01T", [KSP, CW], bf16, isOutput=False)
    u23T = nc.declare_dram_parameter("u23T", [KSP, CW], bf16, isOutput=False)
    v01 = nc.declare_dram_parameter("v01", [KSP, C], bf16, isOutput=False)
    v23 = nc.declare_dram_parameter("v23", [KSP, C], bf16, isOutput=False)
    spk = nc.declare_dram_parameter("spk", [128, 128], bf16, isOutput=False)
    idn = nc.declare_dram_parameter("idn", [128, 128], bf16, isOutput=False)
    ips2_r = nc.declare_dram_parameter("ips2_r", [128, NBLK], f32, isOutput=False)
    xvT = nc.declare_dram_parameter("xvT", [128, CH // 128], f32, isOutput=False)
    y = nc.declare_dram_parameter("y", [CH, CW], f32, isOutput=True)

    imm = lambda v: mybir.ImmediateValue(dtype=mybir.dt.float32, value=v)

    with TileContext(nc) as tc:
        with (
            tc.tile_pool(name="persist", bufs=1) as pp,
            tc.tile_pool(name="work", bufs=1) as wp,
            tc.tile_pool(name="psum", bufs=2, space="PSUM") as psp,
            tc.tile_pool(name="dram", bufs=1, space="DRAM") as dp,
        ):
            u01s = pp.tile([KSP, CW], bf16)
            nc.sync.dma_start(out=u01s[:, :], in_=u01T[:, :])
            u23s = pp.tile([KSP, CW], bf16)
            nc.sync.dma_start(out=u23s[:, :], in_=u23T[:, :])
            v01s = pp.tile([KSP, C], bf16)
            nc.sync.dma_start(out=v01s[:, :], in_=v01[:, :])
            v23s = pp.tile([KSP, C], bf16)
            nc.sync.dma_start(out=v23s[:, :], in_=v23[:, :])
            spk_t = pp.tile([128, 128], bf16)
            nc.sync.dma_start(out=spk_t[:, :], in_=spk[:, :])
            idn_t = pp.tile([128, 128], bf16)
            nc.sync.dma_start(out=idn_t[:, :], in_=idn[:, :])
            ips2 = pp.tile([128, NBLK], f32)
            nc.sync.dma_start(out=ips2[:, :], in_=ips2_r[:, :])
            xvTs = pp.tile([128, CH // 128], f32)
            nc.sync.dma_start(out=xvTs[:, :], in_=xvT[:, :])

            Racc = pp.tile([128, NBLK, NTD], f32)

            def main(n):
                nsl = slice(128 * n, 128 * (n + 1))
                dj = 128 * n   # local diag column (V columns are rolled per core)
                for td in range(NTD):
                    q01 = psp.tile([128, 1024], f32, tag="q01", name="q01")
                    q23 = psp.tile([128, 1024], f32, tag="q23", name="q23")
                    for h in range(2):
                        jc = td * 1024 + h * 512
                        nc.tensor.matmul(
                            q01[:, h * 512:(h + 1) * 512],
                            u01s[:, nsl], v01s[:, jc:jc + 512],
                            start=True, stop=True,
                        )
                    for h in range(2):
                        jc = td * 1024 + h * 512
                        has_diag = jc <= dj < jc + 512
                        nc.tensor.matmul(
                            q23[:, h * 512:(h + 1) * 512],
                            u23s[:, nsl], v23s[:, jc:jc + 512],
                            start=True, stop=not has_diag,
                        )
                        if has_diag:
                            off = h * 512 + (dj - jc)
                            nc.tensor.matmul(
                                q23[:, off:off + 128], idn_t[:, :], spk_t[:, :],
                                start=False, stop=True, skip_group_check=True,
                            )
                    r01 = wp.tile([128, 1024], f32, tag="r01", bufs=3, name="r01")
                    eng = nc.scalar
                    eng.add_instruction(
                        mybir.InstActivation(
                            name=nc.get_next_instruction_name(),
                            func=mybir.ActivationFunctionType.Reciprocal,
                            ins=[eng.lower_ap(q01[:, :]), imm(0.0), imm(1.0),
                                 imm(0.0)],
                            outs=[eng.lower_ap(r01[:, :])],
                        )
                    )
                    junk = wp.tile([128, 1024], mybir.dt.bfloat16, tag="junk",
                                   bufs=2, name="junk")
                    nc.vector._custom_dve(
                        mro, out=junk[:, :], in0=r01[:, :], in1=q23[:, :],
                        s0=RECIP_C0, s1=RECIP_C1,
                        accum_out=Racc[:, n, td:td + 1],
                    )

            def epilogue(g):
                nsl = slice(4 * g, 4 * (g + 1))
                t0 = wp.tile([128, 4], f32, tag="ep0", bufs=2, name="t0")
                nc.vector.tensor_tensor(
                    t0[:, :], Racc[:, nsl, 0], Racc[:, nsl, 1], Alu.add)
                t1 = wp.tile([128, 4], f32, tag="ep1", bufs=2, name="t1")
                nc.vector.tensor_tensor(
                    t1[:, :], Racc[:, nsl, 2], Racc[:, nsl, 3], Alu.add)
                nc.vector.tensor_tensor(t0[:, :], t0[:, :], t1[:, :], Alu.add)
                nc.vector.tensor_tensor(
                    t0[:, :], t0[:, :], ips2[:, nsl], Alu.add)
                rec = wp.tile([128, 4], f32, tag="ep2", bufs=2, name="rec")
                nc.vector.reciprocal(rec[:, :], t0[:, :])
                coef = wp.tile([128, 4], f32, tag="ep3", bufs=2, name="coef")
                nc.vector.tensor_tensor(
                    coef[:, :], rec[:, :], ips2[:, nsl], Alu.mult)
                # transpose coef (128, 4) -> DRAM row (1, 512), then
                # broadcast it back across all 128 partitions
                scrT = dp.tile([1, 512], f32, tag="scrT", bufs=2, name="scrT")
                nc.sync.dma_start(
                    out=scrT.rearrange("a (n p) -> (a p) n", p=128),
                    in_=coef[:, :],
                )
                crowB = wp.tile([128, 512], f32, tag="crowB", bufs=2, name="crowB")
                nc.sync.dma_start(
                    out=crowB[:, :], in_=scrT[0:1, :].broadcast_to([128, 512])
                )
                # y[ch, c] = x[ch] * coef[c] on ACT (per-partition scale),
                # keeping the epilogue off PE/PSUM so the next group's main
                # loop is never blocked on this chain
                for h in range(CH // 128):
                    ysb = wp.tile([128, 512], f32, tag="ysb", bufs=2, name="ysb")
                    nc.scalar.mul(ysb[:, :], crowB[:, :], xvTs[:, h:h + 1])
                    nc.sync.dma_start(
                        out=y[h * 128:(h + 1) * 128, g * 512:(g + 1) * 512],
                        in_=ysb[:, :],
                    )

            def whole():
                for g in range(NG):
                    for n in range(4 * g, 4 * (g + 1)):
                        main(n)
                    epilogue(g)

            if bench_nrep is None:
                whole()
            else:
                with tc.For_i(
                    0, bench_nrep, 1,
                    staggered_reset=True,
                    hint_engines=(mybir.EngineType.DVE,
                                  mybir.EngineType.Activation),
                ):
                    whole()
    nc.finalize()
    return nc


def _get_nc():
    if "nc" not in _cache:
        _cache["nc"] = _build()
    return _cache["nc"]


def _in_maps(x, mu, sig):
    maps = []
    import ml_dtypes
    spike = np.ascontiguousarray(
        (np.eye(128, dtype=np.float32) * np.float32(1e15)).astype(ml_dtypes.bfloat16))
    ident = np.ascontiguousarray(np.eye(128, dtype=np.float32).astype(ml_dtypes.bfloat16))
    for k in range(NCORES):
        b = k // 2
        half = k % 2
        sl = slice(half * CW, (half + 1) * CW)
        mt = (np.asarray(mu[b], np.float32) - np.float32(0.5)).astype(np.float32)
        s = np.asarray(sig[b], np.float32)
        mi = mt[sl]
        si = s[sl]
        a = (si * si + mi * mi).astype(np.float32)
        bv = (np.float32(-2.0) * mi).astype(np.float32)
        one = np.ones_like(a)
        ivec = np.stack([a, bv, one], axis=2)        # (CW, 4, 3)
        jvec = np.stack(
            [np.ones_like(mt), mt, (mt * mt).astype(np.float32)], axis=2
        )                                            # (C, 4, 3)

        def pair_UV(d0, d1):
            U = (ivec[:, d0, :, None] * ivec[:, d1, None, :]).reshape(CW, 9)
            V = (jvec[:, d0, :, None] * jvec[:, d1, None, :]).reshape(C, 9)
            return _split_concat(
                np.ascontiguousarray(U.T.astype(np.float32)),
                np.ascontiguousarray(V.T.astype(np.float32)),
            )

        U01c, V01c = pair_UV(0, 1)
        U23c, V23c = pair_UV(2, 3)
        # rotate j-columns so each core's diagonal lands at local col 128*n
        V01c = np.ascontiguousarray(np.roll(V01c, -half * CW, axis=1))
        V23c = np.ascontiguousarray(np.roll(V23c, -half * CW, axis=1))
        ps2 = (si * si).prod(axis=1, dtype=np.float64)
        ips2 = (1.0 / ps2).astype(np.float32)         # (CW,)
        maps.append(
            {
                "u01T": U01c, "u23T": U23c, "v01": V01c, "v23": V23c,
                "spk": spike, "idn": ident,
                "ips2_r": np.ascontiguousarray(ips2.reshape(NBLK, 128).T),
                "xvT": np.ascontiguousarray(
                    np.asarray(x[b, :, 0], dtype=np.float32).reshape(CH // 128, 128).T
                ),
            }
        )
    return maps


def kernel(x, pi, mu, sig):
    from concourse.bass_utils import run_bass_kernel_spmd

    nc = _get_nc()
    res = run_bass_kernel_spmd(nc, _in_maps(x, mu, sig), list(range(NCORES))).results
    y = np.empty((B, CH, C), np.float32)
    for k in range(NCORES):
        b = k // 2
        half = k % 2
        y[b, :, half * CW:(half + 1) * CW] = res[k]["y"]
    return y


# revision 10
# speedup vs baseline: 1.1668x; 1.1668x over previous
"""Trainium2 Bass kernel for nn_MixtureAttention.

Math: the reference builds a (c,c) pairwise Cauchy-product matrix per batch,
row-normalizes it, and keeps only the diagonal.  `pi` cancels; with
    D[i,j] = prod_d (sig_id^2 + (mu_id - mu_jd)^2)
the kept diagonal is coef[i] = invps2[i] / (invps2[i] + sum_{j!=i} 1/D[i,j])
where invps2 = 1/prod_d sig^2, and y[b,ch,c] = x[b,ch] * coef[b,c].

Kernel strategy (8 cores; core k: batch k//2, c-rows [(k%2)*2048, +2048)):
  - PE: D factorizes per dim-pair into rank-9 bilinear forms in host-computed
    factor matrices (centered coords keep all terms <= 1).  Each pair product
    q01/q23 is ONE bf16 matmul with an exact 3-way hi/mid/lo mantissa split
    folded into the contraction dim (K=9x6=54) -- fp32-grade precision at
    bf16 speed (1 cycle/row).  A bf16 identity "spike" matmul adds
    1e15 to diagonal entries so j=i drops out of the device sum; the exact
    diagonal is re-added in the epilogue from host-computed invps2.
  - ACT: R01 = 1/q01 via Reciprocal (PSUM->SBUF), doubling as PSUM evacuation.
  - DVE: one fused custom op per element: accum += R01 * recip_1NR(q23_psum)
    (bitflip seed + 1 Newton step, ~0.36% max; errors average out in the sum).
  - Epilogue per 512-column group: coef = invps2/(Racc+invps2), transposed via
    a DRAM bounce, then y = x (x) coef as K=1 fp32 outer-product matmuls.
"""

import numpy as np
import re

B, C, D, CH = 4, 4096, 4, 256
NCORES = 8
CW = C // 2            # 2048 c-rows per core (2 cores per batch)
NBLK = CW // 128       # 16 row blocks
KSP = 54               # split contraction: 9 rank-terms x 6 hi/mid/lo pairs
NTD = C // 1024        # 4 j double-tiles per row block
NG = 4                 # epilogue groups (4 row blocks each)

_cache = {}

RECIP_C0 = -0.23549792
RECIP_C1 = 2.0017324


def _split_concat(U, V):
    """U [9,M], V [9,N] fp32 -> K=54 bf16 concatenation.  u = h+m+l is an
    EXACT 3-way bf16 mantissa split (8+8+8 bits); keeping the 6 cross-term
    blocks up to O(2^-24) gives fp32-grade products at bf16 matmul speed."""
    import ml_dtypes

    def split3(x):
        x = np.asarray(x, np.float32)
        h = x.astype(ml_dtypes.bfloat16).astype(np.float32)
        r = (x - h).astype(np.float32)
        m = r.astype(ml_dtypes.bfloat16).astype(np.float32)
        l = (r - m).astype(np.float32).astype(ml_dtypes.bfloat16).astype(np.float32)
        return h, m, l

    Uh, Um, Ul = split3(U)
    Vh, Vm, Vl = split3(V)
    Uc = np.concatenate([Uh, Uh, Um, Uh, Ul, Um], axis=0)
    Vc = np.concatenate([Vh, Vm, Vh, Vl, Vh, Vm], axis=0)
    return (np.ascontiguousarray(Uc.astype(ml_dtypes.bfloat16)),
            np.ascontiguousarray(Vc.astype(ml_dtypes.bfloat16)))


def _get_mulrecip_op():
    """Custom DVE op: out = in0 * recip_1NR(in1); accum_out = row-sum(out)."""
    if "mro" in _cache:
        return _cache["mro"]
    from operator import add
    from concourse import dve_ops as DO
    from concourse.dve_spec import AluOp, Bin, Spec, Src0, Src1, Zero, C0, C1

    name = "MUL_RECIP1NR_ACC_ANT"
    _not = Bin(AluOp.BITWISE_NOT, Src1, Src1)
    _y0 = _not * C0
    _y1 = _y0 * (C1 - Src1 * _y0)

    def _ref(in0, in1, c0, c1, c2):
        nx = (~np.asarray(in1, np.float32).view(np.int32)).view(np.float32)
        y0 = (nx * np.float32(c0)).astype(np.float32)
        y1 = (y0 * (np.float32(c1) - in1 * y0).astype(np.float32)).astype(np.float32)
        b = (in0 * y1).astype(np.float32)
        return b, b.reshape(b.shape[0], -1).sum(axis=-1, keepdims=True)

    spec = Spec(body=Src0 * _y1, accum=add, accum_init=Zero, reference=_ref)
    shas = {}
    for ver in ("v3", "v4"):
        probe = DO.DveOp(name + "_PROBE", spec, subdim=False, uops_sha={})
        if name + "_PROBE" not in DO._SUB_OPCODE_FOR_NAME:
            DO._SUB_OPCODE_FOR_NAME[name + "_PROBE"] = 0x1F
        try:
            probe.compile(ver)
        except ValueError as e:
            m = re.search(r'"(?:v3|v4)"\]="([0-9a-f]+)"', str(e))
            if not m:
                raise
            shas[ver] = m.group(1)
    op = DO.DveOp(name, spec, subdim=False, uops_sha=shas)
    if name not in DO._SUB_OPCODE_FOR_NAME:
        DO.OPS.append(op)
        DO._SUB_OPCODE_FOR_NAME[name] = DO._CUSTOM_DVE_ROW_BASE + len(DO.OPS) - 1
        assert DO._SUB_OPCODE_FOR_NAME[name] < 0x20
    DO.CUSTOM_DVE_SPECS[name] = spec
    _cache["mro"] = op
    return op


def _build(bench_nrep=None, bench_span="full"):
    import concourse.bacc as bacc
    import concourse.mybir as mybir
    from concourse.tile import TileContext

    f32 = mybir.dt.float32
    bf16 = mybir.dt.bfloat16
    Alu = mybir.AluOpType
    mro = _get_mulrecip_op()

    nc = bacc.Bacc(None, target_bir_lowering=False)
    u01T = nc.declare_dram_parameter("u01T", [KSP, CW], bf16, isOutput=False)
    u23T = nc.declare_dram_parameter("u23T", [KSP, CW], bf16, isOutput=False)
    v01 = nc.declare_dram_parameter("v01", [KSP, C], bf16, isOutput=False)
    v23 = nc.declare_dram_parameter("v23", [KSP, C], bf16, isOutput=False)
    spk = nc.declare_dram_parameter("spk", [128, 128], bf16, isOutput=False)
    idn = nc.declare_dram_parameter("idn", [128, 128], bf16, isOutput=False)
    ips2_r = nc.declare_dram_parameter("ips2_r", [128, NBLK], f32, isOutput=False)
    xvT = nc.declare_dram_parameter("xvT", [128, CH // 128], f32, isOutput=False)
    y = nc.declare_dram_parameter("y", [CH, CW], f32, isOutput=True)

    imm = lambda v: mybir.ImmediateValue(dtype=mybir.dt.float32, value=v)

    with TileContext(nc) as tc:
        with (
            tc.tile_pool(name="persist", bufs=1) as pp,
            tc.tile_pool(name="work", bufs=1) as wp,
            tc.tile_pool(name="psum", bufs=2, space="PSUM") as psp,
            tc.tile_pool(name="dram", bufs=1, space="DRAM") as dp,
        ):
            u01s = pp.tile([KSP, CW], bf16)
            nc.sync.dma_start(out=u01s[:, :], in_=u01T[:, :])
            u23s = pp.tile([KSP, CW], bf16)
            nc.sync.dma_start(out=u23s[:, :], in_=u23T[:, :])
            v01s = pp.tile([KSP, C], bf16)
            nc.sync.dma_start(out=v01s[:, :], in_=v01[:, :])
            v23s = pp.tile([KSP, C], bf16)
            nc.sync.dma_start(out=v23s[:, :], in_=v23[:, :])
            spk_t = pp.tile([128, 128], bf16)
            nc.sync.dma_start(out=spk_t[:, :], in_=spk[:, :])
            idn_t = pp.tile([128, 128], bf16)
            nc.sync.dma_start(out=idn_t[:, :], in_=idn[:, :])
            ips2 = pp.tile([128, NBLK], f32)
            nc.sync.dma_start(out=ips2[:, :], in_=ips2_r[:, :])
            xvTs = pp.tile([128, CH // 128], f32)
            nc.sync.dma_start(out=xvTs[:, :], in_=xvT[:, :])

            Racc = pp.tile([128, NBLK, NTD], f32)

            def main(n):
                nsl = slice(128 * n, 128 * (n + 1))
                dj = 128 * n   # local diag column (V columns are rolled per core)
                for tp in range(NTD // 2):
                    # td pair: group matmuls by stationary operand so each
                    # weight load serves 4 consecutive matmuls
                    qa = [psp.tile([128, 1024], f32, tag="q01", name="q01")
                          for _ in range(2)]
                    qb = [psp.tile([128, 1024], f32, tag="q23", name="q23")
                          for _ in range(2)]
                    for i in range(2):
                        td = 2 * tp + i
                        for h in range(2):
                            jc = td * 1024 + h * 512
                            nc.tensor.matmul(
                                qa[i][:, h * 512:(h + 1) * 512],
                                u01s[:, nsl], v01s[:, jc:jc + 512],
                                start=True, stop=True,
                            )
                    for i in range(2):
                        td = 2 * tp + i
                        for h in range(2):
                            jc = td * 1024 + h * 512
                            has_diag = jc <= dj < jc + 512
                            nc.tensor.matmul(
                                qb[i][:, h * 512:(h + 1) * 512],
                                u23s[:, nsl], v23s[:, jc:jc + 512],
                                start=True, stop=not has_diag,
                            )
                            if has_diag:
                                off = h * 512 + (dj - jc)
                                nc.tensor.matmul(
                                    qb[i][:, off:off + 128], idn_t[:, :],
                                    spk_t[:, :],
                                    start=False, stop=True,
                                    skip_group_check=True,
                                )
                    for i in range(2):
                        td = 2 * tp + i
                        r01 = wp.tile([128, 1024], f32, tag="r01", bufs=4,
                                      name="r01")
                        eng = nc.scalar
                        eng.add_instruction(
                            mybir.InstActivation(
                                name=nc.get_next_instruction_name(),
                                func=mybir.ActivationFunctionType.Reciprocal,
                                ins=[eng.lower_ap(qa[i][:, :]), imm(0.0),
                                     imm(1.0), imm(0.0)],
                                outs=[eng.lower_ap(r01[:, :])],
                            )
                        )
                        junk = wp.tile([128, 1024], f32, tag="junk", bufs=3,
                                       name="junk")
                        nc.vector._custom_dve(
                            mro, out=junk[:, :], in0=r01[:, :], in1=qb[i][:, :],
                            s0=RECIP_C0, s1=RECIP_C1,
                            accum_out=Racc[:, n, td:td + 1],
                        )

            def epilogue(g):
                nsl = slice(4 * g, 4 * (g + 1))
                t0 = wp.tile([128, 4], f32, tag="ep0", bufs=2, name="t0")
                nc.vector.tensor_tensor(
                    t0[:, :], Racc[:, nsl, 0], Racc[:, nsl, 1], Alu.add)
                t1 = wp.tile([128, 4], f32, tag="ep1", bufs=2, name="t1")
                nc.vector.tensor_tensor(
                    t1[:, :], Racc[:, nsl, 2], Racc[:, nsl, 3], Alu.add)
                nc.vector.tensor_tensor(t0[:, :], t0[:, :], t1[:, :], Alu.add)
                nc.vector.tensor_tensor(
                    t0[:, :], t0[:, :], ips2[:, nsl], Alu.add)
                rec = wp.tile([128, 4], f32, tag="ep2", bufs=2, name="rec")
                nc.vector.reciprocal(rec[:, :], t0[:, :])
                coef = wp.tile([128, 4], f32, tag="ep3", bufs=2, name="coef")
                nc.vector.tensor_tensor(
                    coef[:, :], rec[:, :], ips2[:, nsl], Alu.mult)
                # transpose coef (128, 4) -> DRAM row (1, 512), then
                # broadcast it back across all 128 partitions
                scrT = dp.tile([1, 512], f32, tag="scrT", bufs=2, name="scrT")
                nc.sync.dma_start(
                    out=scrT.rearrange("a (n p) -> (a p) n", p=128),
                    in_=coef[:, :],
                )
                crowB = wp.tile([128, 512], f32, tag="crowB", bufs=2, name="crowB")
                nc.sync.dma_start(
                    out=crowB[:, :], in_=scrT[0:1, :].broadcast_to([128, 512])
                )
                # y[ch, c] = x[ch] * coef[c] on ACT (per-partition scale),
                # keeping the epilogue off PE/PSUM so the next group's main
                # loop is never blocked on this chain
                for h in range(CH // 128):
                    ysb = wp.tile([128, 512], f32, tag="ysb", bufs=2, name="ysb")
                    nc.scalar.mul(ysb[:, :], crowB[:, :], xvTs[:, h:h + 1])
                    nc.sync.dma_start(
                        out=y[h * 128:(h + 1) * 128, g * 512:(g + 1) * 512],
                        in_=ysb[:, :],
                    )

            def whole():
                for g in range(NG):
                    for n in range(4 * g, 4 * (g + 1)):
                        main(n)
                    epilogue(g)

            if bench_nrep is None:
                whole()
            else:
                with tc.For_i(
                    0, bench_nrep, 1,
                    staggered_reset=True,
                    hint_engines=(mybir.EngineType.DVE,
                                  mybir.EngineType.Activation,
                                  mybir.EngineType.PE),
                ):
                    whole()
    nc.finalize()
    return nc


def _get_nc():
    if "nc" not in _cache:
        _cache["nc"] = _build()
    return _cache["nc"]


def _in_maps(x, mu, sig):
    maps = []
    import ml_dtypes
    spike = np.ascontiguousarray(
        (np.eye(128, dtype=np.float32) * np.float32(1e15)).astype(ml_dtypes.bfloat16))
    ident = np.ascontiguousarray(np.eye(128, dtype=np.float32).astype(ml_dtypes.bfloat16))
    for k in range(NCORES):
        b = k // 2
        half = k % 2
        sl = slice(half * CW, (half + 1) * CW)
        mt = (np.asarray(mu[b], np.float32) - np.float32(0.5)).astype(np.float32)
        s = np.asarray(sig[b], np.float32)
        mi = mt[sl]
        si = s[sl]
        a = (si * si + mi * mi).astype(np.float32)
        bv = (np.float32(-2.0) * mi).astype(np.float32)
        one = np.ones_like(a)
        ivec = np.stack([a, bv, one], axis=2)        # (CW, 4, 3)
        jvec = np.stack(
            [np.ones_like(mt), mt, (mt * mt).astype(np.float32)], axis=2
        )                                            # (C, 4, 3)

        def pair_UV(d0, d1):
            U = (ivec[:, d0, :, None] * ivec[:, d1, None, :]).reshape(CW, 9)
            V = (jvec[:, d0, :, None] * jvec[:, d1, None, :]).reshape(C, 9)
            return _split_concat(
                np.ascontiguousarray(U.T.astype(np.float32)),
                np.ascontiguousarray(V.T.astype(np.float32)),
            )

        U01c, V01c = pair_UV(0, 1)
        U23c, V23c = pair_UV(2, 3)
        # rotate j-columns so each core's diagonal lands at local col 128*n
        V01c = np.ascontiguousarray(np.roll(V01c, -half * CW, axis=1))
        V23c = np.ascontiguousarray(np.roll(V23c, -half * CW, axis=1))
        ps2 = (si * si).prod(axis=1, dtype=np.float64)
        ips2 = (1.0 / ps2).astype(np.float32)         # (CW,)
        maps.append(
            {
                "u01T": U01c, "u23T": U23c, "v01": V01c, "v23": V23c,
                "spk": spike, "idn": ident,
                "ips2_r": np.ascontiguousarray(ips2.reshape(NBLK, 128).T),
                "xvT": np.ascontiguousarray(
                    np.asarray(x[b, :, 0], dtype=np.float32).reshape(CH // 128, 128).T
                ),
            }
        )
    return maps


def kernel(x, pi, mu, sig):
    from concourse.bass_utils import run_bass_kernel_spmd

    nc = _get_nc()
    res = run_bass_kernel_spmd(nc, _in_maps(x, mu, sig), list(range(NCORES))).results
    y = np.empty((B, CH, C), np.float32)
    for k in range(NCORES):
        b = k // 2
        half = k % 2
        y[b, :, half * CW:(half + 1) * CW] = res[k]["y"]
    return y
